# revision 20
# baseline (speedup 1.0000x reference)
"""Trainium2 Bass kernel for the nn_Detect head (3-level YOLO-style decode).

Strategy: data-parallel over batch (8 images -> 8 NeuronCores). The key
algebraic move: the reference has NO nonlinearity between the 3x3 conv and
the 1x1 head conv, so they compose on the host into a single 3x3 conv with
only 51 output channels (3 anchors x 17) -- a 17x FLOP reduction vs the
baseline's full C->C conv + 1x1 head.

Per core:
  - composed 3x3 conv as 9-tap shifted matmuls (config: weights stationary
    [cin=128, 51], pixels streaming), fp16 in / fp32 PSUM out. Since M=51
    wastes half the PE columns, the (tap, cin-chunk) units are split
    even/odd across PE column-groups 0-1 and 64-115 (2-way column tiling,
    inferred from the PSUM slice base partition) so two streams run
    concurrently in the 128x128 array.
  - PSUM [51|51, px] -> SBUF fp16 h buffer (scalar/vector copy, + composed
    bias when nonzero), then pixel-major transpose via PE matmuls with a
    host-provided identity as the moving operand: tp[px,51] = hA.T + hB.T
    (two accumulating matmuls at row-positions 0 and 64 -- row tiling).
  - decode (sigmoid/grid/anchor/dims/orient-norm) identical to px-major
    decode on [128, chunk, 51] PSUM tiles, writing per-(level, anchor)
    staging tiles; orientation L2-normalize deferred to one Sqrt + DVE
    reciprocal pass per level; output stores deferred one level to keep
    the 68B-granular flood off the conv's weight stream.
  - ~16 warm-up matmuls on the first weight tile during the initial input
    DMA window flip the PE HAM clock-gate to 8/8 before the real conv.
Host side composes/packs/pads/transposes all inputs.
"""

import numpy as np
import ml_dtypes

BS = 8
NCORES = 8
NO2D = 8
NO3D = 9
NOUT = 17
NROWS = 25200
NCH = 51  # 3 anchors x 17 composed conv output channels

# (C, H, W, stride, W2pad, slab_rows)
LEVELS = [
    (256, 80, 80, 8.0, 88, [6] * 13 + [2]),
    (512, 40, 40, 16.0, 48, [12] * 3 + [4]),
    (1024, 20, 20, 32.0, 24, [20]),
]
ANCHORS = np.array(
    [
        [[10, 13], [16, 30], [33, 23]],
        [[30, 61], [62, 45], [59, 119]],
        [[116, 90], [156, 198], [373, 326]],
    ],
    np.float32,
)

_S = [H * W for (_, H, W, _, _, _) in LEVELS]          # 6400, 1600, 400
_NCHK = [(s + 127) // 128 for s in _S]                 # 50, 13, 4
_ROW0 = [0, 3 * _S[0], 3 * _S[0] + 3 * _S[1]]          # level row offsets
_NBOFF = []
_off = 0
for _l in range(3):
    for _a in range(3):
        _NBOFF.append(_off)
        _off += _NCHK[_l]
_NBTOT = _off                                           # 201

_PROGRAM_CACHE = {}


def _groups(nchunks, g=10):
    out = []
    k0 = 0
    while k0 < nchunks:
        gn = min(g, nchunks - k0)
        out.append((k0, gn))
        k0 += gn
    return out


def _build_program(bias_flags):
    import concourse.mybir as mybir
    import concourse.tile as tile
    from concourse import bacc

    nc = bacc.Bacc(None)
    f32 = mybir.dt.float32
    f16 = mybir.dt.float16

    fps = []
    was = []
    grids = []
    for l, (C, H, W, _, _, _) in enumerate(LEVELS):
        Q = C // 128
        W2 = LEVELS[l][4]
        fps.append(nc.declare_dram_parameter(f"f{l}p", [128, Q, H + 2, W2], f16, isOutput=False))
        was.append(nc.declare_dram_parameter(f"wa{l}", [128, Q, 9, NCH], f16, isOutput=False))
        grids.append(nc.declare_dram_parameter(f"grid{l}", [128, _NCHK[l], 2], f32, isOutput=False))
    anch = nc.declare_dram_parameter("anch", [128, 3, 3, 2], f32, isOutput=False)
    ident = nc.declare_dram_parameter("ident", [128, NCH], f16, isOutput=False)
    bcs = {}
    for l in range(3):
        if bias_flags[l]:
            bcs[l] = nc.declare_dram_parameter(f"bc{l}", [128, 1], f32, isOutput=False)
    out = nc.declare_dram_parameter("out", [NROWS, NOUT], f32, isOutput=True)

    with tile.TileContext(nc) as tc:
        from contextlib import ExitStack

        with ExitStack() as ctx:
            cpool = ctx.enter_context(tc.tile_pool(name="consts", bufs=1))
            spool = ctx.enter_context(tc.tile_pool(name="stage", bufs=1))
            ipool = ctx.enter_context(tc.tile_pool(name="inbuf", bufs=1))
            wpool = ctx.enter_context(tc.tile_pool(name="wconv", bufs=1))
            hpool = ctx.enter_context(tc.tile_pool(name="hbuf", bufs=1))
            cppool = ctx.enter_context(tc.tile_pool(name="cpsum", bufs=2, space="PSUM"))
            tppool = ctx.enter_context(tc.tile_pool(name="tpsum", bufs=2, space="PSUM"))
            wmpool = ctx.enter_context(tc.tile_pool(name="wmpsum", bufs=1, space="PSUM"))
            scpool = ctx.enter_context(tc.tile_pool(name="scratch", bufs=2))

            sig = mybir.ActivationFunctionType.Sigmoid
            mult = mybir.AluOpType.mult
            add = mybir.AluOpType.add

            # ---- constants (issued first on the sync DMA queue) ----
            idt = cpool.tile([128, NCH], f16, tag="ident")
            nc.sync.dma_start(idt[:], ident[:])
            gts = []
            for l in range(3):
                gt = cpool.tile([128, _NCHK[l], 2], f32, tag=f"grid{l}")
                nc.sync.dma_start(gt[:], grids[l][:])
                gts.append(gt)
            ancht = cpool.tile([128, 3, 3, 2], f32)
            nc.sync.dma_start(ancht[:], anch[:])
            epst = cpool.tile([128, 1], f32)
            nc.vector.memset(epst[:], 1e-24)
            bct = {}
            for l, p in bcs.items():
                t = cpool.tile([128, 1], f32, tag=f"bc{l}")
                nc.sync.dma_start(t[:], p[:])
                bct[l] = t

            # ---- conv weights (small; wa0 first so warm-up can start) ----
            wat = []
            for l in range(3):
                Q = LEVELS[l][0] // 128
                wt = wpool.tile([128, Q, 9, NCH], f16, tag=f"wa{l}")
                wat.append(wt)
            nc.sync.dma_start(wat[0][:], was[0][:])

            # ---- HAM warm-up: keep PE busy during the first input DMAs ----
            import os
            if not os.environ.get("BASSK_NOWARM"):
                wmp = wmpool.tile([128, 9 * NCH], f32, tag="warm")
                for i in range(16):
                    nc.tensor.matmul(
                        wmp[0:NCH, :],
                        wat[0][:, 0, 0, :],
                        wat[0][:, 0].rearrange("p t o -> p (t o)"),
                        start=True,
                        stop=True,
                    )

            # ---- staging + norm buffers (persist to end) ----
            st = [
                [
                    spool.tile([128, _NCHK[l], NOUT], f32, tag=f"st{l}{a}", name=f"st{l}{a}")
                    for a in range(3)
                ]
                for l in range(3)
            ]
            nb = spool.tile([128, _NBTOT, 2], f32)

            # ---- input feature DMAs, slab-aligned chunks ----
            inb = []
            for l, (C, H, W, _, W2, slab_rows) in enumerate(LEVELS):
                Q = C // 128
                it = ipool.tile([128, Q, H + 2, W2], f16, tag=f"inb{l}", name=f"inb{l}")
                inb.append(it)
                if l > 0:
                    nc.sync.dma_start(wat[l][:], was[l][:])
                import os
                if os.environ.get("BASSK_CHUNKDMA"):
                    for q in range(Q):
                        nc.sync.dma_start(it[:, q], fps[l][:, q])
                else:
                    r0 = 0
                    for si, rows in enumerate(slab_rows):
                        # chunk covers padded rows [pr0, pr1): slab si consumes
                        # padded rows r0 .. r0+rows+1, covered by chunks <= si
                        pr0 = 0 if si == 0 else r0 + 2
                        pr1 = r0 + rows + 2
                        for q in range(Q):
                            nc.sync.dma_start(it[:, q, pr0:pr1], fps[l][:, q, pr0:pr1])
                        r0 += rows

            import os as _os
            _lvls = _os.environ.get("BASSK_LVLS", "012")
            _serial = bool(_os.environ.get("BASSK_SERIAL"))
            _stage = _os.environ.get("BASSK_STAGE", "all")  # conv|tp|dec|all
            copy_ctr = 0
            pending_tails = []
            for l, (C, H, W, stride, W2, slab_rows) in enumerate(LEVELS):
                Q = C // 128
                S = H * W
                if str(l) not in _lvls:
                    continue

                if pending_tails:
                    # flush previous level's fixup+stores under this level's
                    # compute window so the 68B-granular store flood never
                    # competes with a cold pipeline
                    for fn in pending_tails:
                        fn()
                    pending_tails.clear()

                # ---- composed 3x3 conv: (tap, q) units split even/odd over
                # PE column groups 0-50 / 64-114 (concurrent streams) ----
                units = [(t, q) for t in range(9) for q in range(Q)]
                SP = _NCHK[l] * 128  # padded so transposes always span 128 px
                h = hpool.tile([128, SP], f16, tag=f"h{l}", name=f"h{l}")
                h2 = None
                if not _serial:
                    h2 = hpool.tile([128, SP], f16, tag=f"h2{l}", name=f"h2{l}")
                if SP > S:
                    nc.vector.memset(h[:, S:SP], 0.0)
                r0 = 0
                for rows in slab_rows:
                    N = rows * W
                    # separate PSUM banks per PE column-group stream: the
                    # accumulation-group zero-region is bank-granular, so the
                    # two interleaved groups may not share a bank
                    blka = cppool.tile([128, 512], f32, tag="cblka", name="cblka")
                    blkb = cppool.tile([128, 512], f32, tag="cblkb", name="cblkb")
                    blks = (blka, blkb)
                    nu = len(units)
                    if _serial:
                        for i, (t, q) in enumerate(units):
                            ty, tx = divmod(t, 3)
                            rhs = inb[l][:, q, r0 + ty : r0 + ty + rows, tx : tx + W]
                            nc.tensor.matmul(
                                blka[0:NCH, :N],
                                wat[l][:, q, t, :],
                                rhs,
                                start=(i == 0),
                                stop=(i == nu - 1),
                            )
                    else:
                        for i in range(nu // 2):
                            for g, (t, q) in ((0, units[2 * i]), (1, units[2 * i + 1])):
                                ty, tx = divmod(t, 3)
                                rhs = inb[l][:, q, r0 + ty : r0 + ty + rows, tx : tx + W]
                                p0 = 64 * g
                                nc.tensor.matmul(
                                    blks[g][p0 : p0 + NCH, :N],
                                    wat[l][:, q, t, :],
                                    rhs,
                                    start=(i == 0),
                                    stop=(i == nu // 2 - 1),
                                )
                    px0 = r0 * W
                    bt = bct.get(l)
                    for g, p0 in ((0, 0),) if _serial else ((0, 0), (1, 64)):
                        dst = h[p0 : p0 + NCH, px0 : px0 + N]
                        src = blks[g][p0 : p0 + NCH, :N]
                        if bt is not None and g == 0:
                            nc.vector.tensor_scalar(
                                dst, src, 1.0, bt[p0 : p0 + NCH, 0:1], mult, add
                            )
                        elif copy_ctr % 2 == 0:
                            nc.vector.tensor_copy(dst, src)
                        else:
                            nc.scalar.copy(dst, src)
                        copy_ctr += 1
                    if not _serial:
                        # bring the column-group-B half down to partitions
                        # 0-50 (partition-shifting local DMA), then fold it in
                        nc.scalar.dma_start(
                            h2[0:NCH, px0 : px0 + N], h[64 : 64 + NCH, px0 : px0 + N]
                        )
                        nc.vector.tensor_tensor(
                            h[0:NCH, px0 : px0 + N],
                            h[0:NCH, px0 : px0 + N],
                            h2[0:NCH, px0 : px0 + N],
                            add,
                        )
                    r0 += rows

                # ---- transpose to pixel-major + decode, per 10-chunk group ----
                if _stage == "conv":
                    continue
                for (k0, gn) in _groups(_NCHK[l]):
                    tpt = tppool.tile([128, 10, NCH], f32, tag="tp")
                    for gi in range(gn):
                        px0 = (k0 + gi) * 128
                        nc.tensor.matmul(
                            tpt[:, gi, :],
                            h[0:NCH, px0 : px0 + 128],
                            idt[0:NCH, :],
                            start=True,
                            stop=True,
                        )
                    if _stage == "tp":
                        continue
                    for a in range(3):
                        sta = st[l][a]
                        cols = sta[:, k0 : k0 + gn, :]
                        c0 = NOUT * a
                        # h2: sigmoid all 8 channels
                        nc.scalar.activation(cols[:, :, 0:NO2D], tpt[:, :gn, c0 : c0 + NO2D], sig)
                        # xy: sig*2s + (grid-0.5)*s
                        nc.vector.tensor_scalar_mul(cols[:, :, 0:2], cols[:, :, 0:2], 2.0 * stride)
                        nc.vector.tensor_tensor(cols[:, :, 0:2], cols[:, :, 0:2], gts[l][:, k0 : k0 + gn, :], add)
                        # wh: (2 sig)^2 A = sig^2 * 4A
                        nc.vector.tensor_tensor(cols[:, :, 2:4], cols[:, :, 2:4], cols[:, :, 2:4], mult)
                        nc.vector.tensor_tensor(
                            cols[:, :, 2:4], cols[:, :, 2:4],
                            ancht[:, l, a, :][:, None, :].to_broadcast([128, gn, 2]), mult,
                        )
                        # h3 bins+orient raw copy
                        nc.vector.tensor_copy(cols[:, :, 8:14], tpt[:, :gn, c0 + 8 : c0 + 14])
                        # orient norm^2 -> norm buffer
                        sqt = scpool.tile([128, 10, 4], f32, tag="sqt")
                        nc.vector.tensor_tensor(sqt[:, :gn, :], cols[:, :, 10:14], cols[:, :, 10:14], mult)
                        sq4 = sqt[:, :gn, :].rearrange("p g (j t) -> p g j t", t=2)
                        noff = _NBOFF[l * 3 + a]
                        nc.vector.tensor_tensor(
                            nb[:, noff + k0 : noff + k0 + gn, :], sq4[:, :, :, 0], sq4[:, :, :, 1], add
                        )
                        # dims: sigmoid * 2 - 1
                        nc.scalar.activation(cols[:, :, 14:17], tpt[:, :gn, c0 + 14 : c0 + 17], sig)
                        nc.vector.tensor_scalar(cols[:, :, 14:17], cols[:, :, 14:17], 2.0, -1.0, mult, add)

                # orientation normalize + output DMA for this level; deferred
                # into the next level's compute window
                def _emit_tail(l=l, S=S):
                    loff = _NBOFF[l * 3]
                    nbl = nb[:, loff : loff + 3 * _NCHK[l], :]
                    nc.scalar.activation(nbl, nbl, mybir.ActivationFunctionType.Sqrt, bias=epst[:])
                    nc.vector.reciprocal(nbl, nbl)
                    kfull, rem = divmod(S, 128)
                    for a in range(3):
                        noff = _NBOFF[l * 3 + a]
                        ori = st[l][a][:, :, 10:14].rearrange("p k (j t) -> p k j t", t=2)
                        rinv = nb[:, noff : noff + _NCHK[l], :][:, :, :, None].to_broadcast(
                            [128, _NCHK[l], 2, 2]
                        )
                        nc.vector.tensor_tensor(ori, ori, rinv, mult)
                        row0 = _ROW0[l] + a * S
                        half = (kfull + 1) // 2
                        for c0, c1 in ((0, half), (half, kfull)):
                            if c1 > c0:
                                nc.gpsimd.dma_start(
                                    out[row0 + c0 * 128 : row0 + c1 * 128, :].rearrange(
                                        "(k p) c -> p k c", p=128
                                    ),
                                    st[l][a][:, c0:c1, :],
                                )
                        if rem:
                            nc.gpsimd.dma_start(
                                out[row0 + kfull * 128 : row0 + S, :],
                                st[l][a][:rem, kfull, :],
                            )

                if _stage != "dec":
                    pending_tails.append(_emit_tail)
                if l == 2:
                    for fn in pending_tails:
                        fn()
                    pending_tails.clear()

    nc.finalize()
    return nc


def _pack_inputs(inputs):
    """Host-side packing: compose 1x1 heads into the 3x3 convs, pad
    activations, cast everything the PE touches to fp16."""
    f16 = ml_dtypes.float16 if hasattr(ml_dtypes, "float16") else np.float16
    f16 = np.float16
    shared = {}
    percore = [dict() for _ in range(BS)]
    bias_flags = []
    for l, (C, H, W, stride, W2, _) in enumerate(LEVELS):
        Q = C // 128
        S = H * W
        f = np.asarray(inputs[f"f{l}"])
        fpad = np.zeros((BS, C, H + 2, W2), f16)
        fpad[:, :, 1 : H + 1, 1 : W + 1] = f.astype(f16)
        fp = np.ascontiguousarray(
            fpad.reshape(BS, Q, 128, H + 2, W2).transpose(0, 2, 1, 3, 4)
        )
        for bcore in range(BS):
            percore[bcore][f"f{l}p"] = fp[bcore]

        # compose the 1x1 head into the 3x3 conv: Wc[o,i,ky,kx] =
        # sum_c wb[o,c] * wa[c,i,ky,kx]; channel order per anchor a is
        # [8 x 2d-head | 9 x 3d-head] matching the NOUT=17 staging layout
        w2a = np.asarray(inputs[f"w2a{l}"]).astype(np.float64)
        w2b = np.asarray(inputs[f"w2b{l}"])[:, :, 0, 0].astype(np.float64)
        w3a = np.asarray(inputs[f"w3a{l}"]).astype(np.float64)
        w3b = np.asarray(inputs[f"w3b{l}"])[:, :, 0, 0].astype(np.float64)
        wc2 = np.einsum("oc,cikl->oikl", w2b, w2a)  # [24, C, 3, 3]
        wc3 = np.einsum("oc,cikl->oikl", w3b, w3a)  # [27, C, 3, 3]
        wc = np.zeros((NCH, C, 3, 3), np.float64)
        for a in range(3):
            wc[a * NOUT : a * NOUT + 8] = wc2[a * 8 : a * 8 + 8]
            wc[a * NOUT + 8 : a * NOUT + 17] = wc3[a * 9 : a * 9 + 9]
        shared[f"wa{l}"] = np.ascontiguousarray(
            wc.reshape(NCH, Q, 128, 9).transpose(2, 1, 3, 0).astype(f16)
        )

        # composed bias: bc[o] = wb[o,:] @ ba + bb[o]
        b2a = np.asarray(inputs[f"b2a{l}"]).astype(np.float64)
        b2b = np.asarray(inputs[f"b2b{l}"]).astype(np.float64)
        b3a = np.asarray(inputs[f"b3a{l}"]).astype(np.float64)
        b3b = np.asarray(inputs[f"b3b{l}"]).astype(np.float64)
        bc2 = w2b @ b2a + b2b
        bc3 = w3b @ b3a + b3b
        bc = np.zeros((NCH,), np.float64)
        for a in range(3):
            bc[a * NOUT : a * NOUT + 8] = bc2[a * 8 : a * 8 + 8]
            bc[a * NOUT + 8 : a * NOUT + 17] = bc3[a * 9 : a * 9 + 9]
        nz = bool(np.any(bc != 0))
        bias_flags.append(nz)
        if nz:
            bcp = np.zeros((128, 1), np.float32)
            bcp[0:NCH, 0] = bc
            bcp[64 : 64 + NCH, 0] = bc
            shared[f"bc{l}"] = bcp

        px = np.arange(_NCHK[l] * 128, dtype=np.float32)
        gx = np.where(px < S, px % W, 0.0).astype(np.float32)
        gy = np.where(px < S, px // W, 0.0).astype(np.float32)
        g = np.stack([(gx - 0.5) * stride, (gy - 0.5) * stride], -1)
        shared[f"grid{l}"] = np.ascontiguousarray(
            g.reshape(_NCHK[l], 128, 2).transpose(1, 0, 2)
        )

    shared["anch"] = np.ascontiguousarray(
        np.broadcast_to(4.0 * ANCHORS[None], (128, 3, 3, 2)).astype(np.float32)
    )
    idm = np.zeros((128, NCH), f16)
    idm[np.arange(NCH), np.arange(NCH)] = 1
    idm[64 + np.arange(NCH), np.arange(NCH)] = 1
    shared["ident"] = idm

    in_maps = []
    for bcore in range(BS):
        m = dict(shared)
        m.update(percore[bcore])
        in_maps.append(m)
    return in_maps, tuple(bias_flags)


def _get_program(bias_flags):
    if bias_flags not in _PROGRAM_CACHE:
        _PROGRAM_CACHE[bias_flags] = _build_program(bias_flags)
    return _PROGRAM_CACHE[bias_flags]


def _run(inputs, trace=False):
    from concourse.bass_utils import run_bass_kernel_spmd

    in_maps, bias_flags = _pack_inputs(inputs)
    nc = _get_program(bias_flags)
    res = run_bass_kernel_spmd(
        nc, in_maps, core_ids=list(range(NCORES)), trace=trace
    )
    outp = np.stack([res.results[i]["out"] for i in range(NCORES)]).astype(np.float32)
    return outp, res


def kernel(**inputs) -> np.ndarray:
    outp, _ = _run(inputs, trace=False)
    return outp


# revision 22
# speedup vs baseline: 1.3731x; 1.3731x over previous
"""Trainium2 Bass kernel for the nn_Detect head (3-level YOLO-style decode).

Strategy: data-parallel over batch (8 images -> 8 NeuronCores). The key
algebraic move: the reference has NO nonlinearity between the 3x3 conv and
the 1x1 head conv, so they compose on the host into a single 3x3 conv with
only 51 output channels (3 anchors x 17) -- a 17x FLOP reduction vs the
baseline's full C->C conv + 1x1 head.

Per core:
  - composed 3x3 conv as 9-tap shifted matmuls (weights stationary
    [cin=128, 51], pixels streaming), fp16 in / fp32 PSUM. Since M=51 wastes
    PE columns, the (tap, cin-chunk) units are split even/odd across PE
    column groups 0-50 / 64-114 (2-way column tiling, inferred from the PSUM
    slice base partition) so two streams run concurrently in the array.
    The two streams accumulate in separate PSUM banks.
  - PSUM -> SBUF fp16 h buffer; the column-group-B half is folded into
    partitions 0-50 by a partition-shifting local DMA + one DVE add per
    level (lane-locked engines cannot cross partitions; repeated matmul
    pairs mixing row-positions 0/64 hard-fault the HW).
  - pixel-major transpose via PE matmuls with identity moving operand,
    using a *strided* stationary read (pixel index = p*nch + k) so the
    final output rows are contiguous per partition -- output stores become
    ~1-3.4KB descriptors instead of a 68B-granular flood.
  - transposes+decode of level l are deferred until after level l+1's conv
    is issued, hiding the fold DMA latency off the PE critical path.
  - decode (sigmoid/grid/anchor/dims/orient-norm) on vector/scalar engines
    into per-(level, anchor) staging tiles; orient L2-normalize deferred to
    one Sqrt + reciprocal pass per level.
  - DMA discipline: one big load per level (L0 split in two y-halves), all
    small constants packed into one [128, 206] f32 blob, loads on the sync
    HWDGE ring, stores on the scalar HWDGE ring, folds on gpsimd SWDGE.
  - 16 warm-up matmuls on the first weight tile flip the PE HAM clock gate
    to 8/8 during the initial input DMA window.
Host side composes/packs/pads/casts all inputs.
"""

import numpy as np

BS = 8
NCORES = 8
NO2D = 8
NO3D = 9
NOUT = 17
NROWS = 25200
NCH = 51  # 3 anchors x 17 composed conv output channels

# (C, H, W, stride, W2pad, slab_rows)
LEVELS = [
    (256, 80, 80, 8.0, 88, [6] * 13 + [2]),
    (512, 40, 40, 16.0, 48, [12] * 3 + [4]),
    (1024, 20, 20, 32.0, 24, [20]),
]
ANCHORS = np.array(
    [
        [[10, 13], [16, 30], [33, 23]],
        [[30, 61], [62, 45], [59, 119]],
        [[116, 90], [156, 198], [373, 326]],
    ],
    np.float32,
)

_S = [H * W for (_, H, W, _, _, _) in LEVELS]          # 6400, 1600, 400
_NCHK = [(s + 127) // 128 for s in _S]                 # 50, 13, 4
_ROW0 = [0, 3 * _S[0], 3 * _S[0] + 3 * _S[1]]          # level row offsets
_NBOFF = []
_off = 0
for _l in range(3):
    for _a in range(3):
        _NBOFF.append(_off)
        _off += _NCHK[_l]
_NBTOT = _off                                           # 201

# const blob layout (f32 columns)
_CB_GRID = [0, 2 * _NCHK[0], 2 * (_NCHK[0] + _NCHK[1])]
_CB_ANCH = 2 * (_NCHK[0] + _NCHK[1] + _NCHK[2])         # 134
_CB_IDENT = _CB_ANCH + 18                               # 152
_CB_BIAS = _CB_IDENT + NCH                              # 203
_CB_TOT = _CB_BIAS + 3                                  # 206

_PROGRAM_CACHE = {}


def _groups(nchunks, g=10):
    out = []
    k0 = 0
    while k0 < nchunks:
        gn = min(g, nchunks - k0)
        out.append((k0, gn))
        k0 += gn
    return out


def _build_program(bias_flags):
    import concourse.mybir as mybir
    import concourse.tile as tile
    from concourse import bacc

    nc = bacc.Bacc(None)
    f32 = mybir.dt.float32
    f16 = mybir.dt.float16

    fps = []
    was = []
    for l, (C, H, W, _, W2, _) in enumerate(LEVELS):
        Q = C // 128
        fps.append(nc.declare_dram_parameter(f"f{l}p", [128, Q, H + 2, W2], f16, isOutput=False))
        was.append(nc.declare_dram_parameter(f"wa{l}", [128, Q, 9, NCH], f16, isOutput=False))
    cblob = nc.declare_dram_parameter("cblob", [128, _CB_TOT], f32, isOutput=False)
    out = nc.declare_dram_parameter("out", [NROWS, NOUT], f32, isOutput=True)

    with tile.TileContext(nc) as tc:
        from contextlib import ExitStack

        with ExitStack() as ctx:
            cpool = ctx.enter_context(tc.tile_pool(name="consts", bufs=1))
            spool = ctx.enter_context(tc.tile_pool(name="stage", bufs=1))
            ipool = ctx.enter_context(tc.tile_pool(name="inbuf", bufs=1))
            wpool = ctx.enter_context(tc.tile_pool(name="wconv", bufs=1))
            hpool = ctx.enter_context(tc.tile_pool(name="hbuf", bufs=1))
            cppool = ctx.enter_context(tc.tile_pool(name="cpsum", bufs=2, space="PSUM"))
            tppool = ctx.enter_context(tc.tile_pool(name="tpsum", bufs=2, space="PSUM"))
            wmpool = ctx.enter_context(tc.tile_pool(name="wmpsum", bufs=1, space="PSUM"))
            scpool = ctx.enter_context(tc.tile_pool(name="scratch", bufs=2))

            sig = mybir.ActivationFunctionType.Sigmoid
            mult = mybir.AluOpType.mult
            add = mybir.AluOpType.add

            # ---- conv weights for level 0 first (warm-up dependency) ----
            wat = []
            for l in range(3):
                Q = LEVELS[l][0] // 128
                wat.append(wpool.tile([128, Q, 9, NCH], f16, tag=f"wa{l}", name=f"wa{l}"))
            nc.sync.dma_start(wat[0][:], was[0][:])

            # ---- packed constants: one DMA ----
            cbt = cpool.tile([128, _CB_TOT], f32, tag="cblob")
            nc.sync.dma_start(cbt[:], cblob[:])
            gts = [
                cbt[:, _CB_GRID[l] : _CB_GRID[l] + 2 * _NCHK[l]].rearrange(
                    "p (k t) -> p k t", t=2
                )
                for l in range(3)
            ]
            ancht = cbt[:, _CB_ANCH : _CB_ANCH + 18].rearrange(
                "p (l a t) -> p l a t", l=3, a=3
            )
            idt = cpool.tile([128, NCH], f16, tag="ident")
            nc.vector.tensor_copy(idt[:], cbt[:, _CB_IDENT : _CB_IDENT + NCH])
            epst = cpool.tile([128, 1], f32)
            nc.vector.memset(epst[:], 1e-24)

            # ---- HAM warm-up: keep PE busy during the first input DMAs ----
            import os as _os
            if not _os.environ.get("BASSK_NOWARM"):
                wmp = wmpool.tile([128, 9 * NCH], f32, tag="warm")
                for i in range(16):
                    nc.tensor.matmul(
                        wmp[0:NCH, :],
                        wat[0][:, 0, 0, :],
                        wat[0][:, 0].rearrange("p t o -> p (t o)"),
                        start=True,
                        stop=True,
                    )

            # ---- staging + norm buffers (persist to end) ----
            st = [
                [
                    spool.tile([128, _NCHK[l], NOUT], f32, tag=f"st{l}{a}", name=f"st{l}{a}")
                    for a in range(3)
                ]
                for l in range(3)
            ]
            nb = spool.tile([128, _NBTOT, 2], f32)

            # ---- input feature DMAs: few big transfers ----
            inb = []
            for l, (C, H, W, _, W2, slab_rows) in enumerate(LEVELS):
                Q = C // 128
                it = ipool.tile([128, Q, H + 2, W2], f16, tag=f"inb{l}", name=f"inb{l}")
                inb.append(it)
                if l > 0:
                    nc.sync.dma_start(wat[l][:], was[l][:])
                if l == 0:
                    hh = (H + 2) // 2
                    nc.sync.dma_start(it[:, :, 0:hh], fps[l][:, :, 0:hh])
                    nc.sync.dma_start(it[:, :, hh:], fps[l][:, :, hh:])
                else:
                    nc.sync.dma_start(it[:], fps[l][:])

            _lvls = _os.environ.get("BASSK_LVLS", "012")
            _stage = _os.environ.get("BASSK_STAGE", "all")  # conv|dec|all
            copy_ctr = 0
            pending = []  # deferred (transpose+decode) closures
            pending_tails = []  # deferred (orient-fix + store) closures

            for l, (C, H, W, stride, W2, slab_rows) in enumerate(LEVELS):
                Q = C // 128
                S = H * W
                if str(l) not in _lvls:
                    continue

                # ---- composed 3x3 conv: (tap, q) units split even/odd over
                # PE column groups 0-50 / 64-114 in separate PSUM banks ----
                units = [(t, q) for t in range(9) for q in range(Q)]
                nchk = _NCHK[l]
                SP = nchk * 128
                h = hpool.tile([128, SP], f16, tag=f"h{l}", name=f"h{l}")
                h2 = hpool.tile([128, SP], f16, tag=f"h2{l}", name=f"h2{l}")
                if SP > S:
                    nc.vector.memset(h[:, S:SP], 0.0)
                r0 = 0
                for rows in slab_rows:
                    N = rows * W
                    blka = cppool.tile([128, 512], f32, tag="cblka", name="cblka")
                    blkb = cppool.tile([128, 512], f32, tag="cblkb", name="cblkb")
                    blks = (blka, blkb)
                    nu = len(units)
                    for i in range(nu // 2):
                        for g, (t, q) in ((0, units[2 * i]), (1, units[2 * i + 1])):
                            ty, tx = divmod(t, 3)
                            rhs = inb[l][:, q, r0 + ty : r0 + ty + rows, tx : tx + W]
                            p0 = 64 * g
                            nc.tensor.matmul(
                                blks[g][p0 : p0 + NCH, :N],
                                wat[l][:, q, t, :],
                                rhs,
                                start=(i == 0),
                                stop=(i == nu // 2 - 1),
                            )
                    px0 = r0 * W
                    has_bias = bias_flags[l]
                    for g, p0 in ((0, 0), (1, 64)):
                        dst = h[p0 : p0 + NCH, px0 : px0 + N]
                        src = blks[g][p0 : p0 + NCH, :N]
                        if has_bias and g == 0:
                            nc.vector.tensor_scalar(
                                dst, src, 1.0,
                                cbt[0:NCH, _CB_BIAS + l : _CB_BIAS + l + 1],
                                mult, add,
                            )
                        elif copy_ctr % 2 == 0:
                            nc.vector.tensor_copy(dst, src)
                        else:
                            nc.scalar.copy(dst, src)
                        copy_ctr += 1
                    r0 += rows

                # fold column-group-B half down to partitions 0-50: one
                # partition-shifting local DMA + one DVE add per level
                nc.gpsimd.dma_start(h2[0:NCH, 0:S], h[64 : 64 + NCH, 0:S])
                nc.vector.tensor_tensor(
                    h[0:NCH, 0:S], h[0:NCH, 0:S], h2[0:NCH, 0:S], add
                )

                def _trans_decode(l=l, S=S, stride=stride, h=h, nchk=nchk):
                    if _stage == "conv":
                        return
                    # strided pixel view: pixel index = p*nchk + k
                    hv = h.rearrange("c (p k) -> c p k", k=nchk)
                    for (k0, gn) in _groups(nchk):
                        tpt = tppool.tile([128, 10, NCH], f32, tag="tp")
                        for gi in range(gn):
                            nc.tensor.matmul(
                                tpt[:, gi, :],
                                hv[0:NCH, :, k0 + gi],
                                idt[0:NCH, :],
                                start=True,
                                stop=True,
                            )
                        for a in range(3):
                            sta = st[l][a]
                            cols = sta[:, k0 : k0 + gn, :]
                            c0 = NOUT * a
                            # h2d: sigmoid all 8 channels
                            nc.scalar.activation(cols[:, :, 0:NO2D], tpt[:, :gn, c0 : c0 + NO2D], sig)
                            # xy: sig*2s + (grid-0.5)*s
                            nc.vector.tensor_scalar_mul(cols[:, :, 0:2], cols[:, :, 0:2], 2.0 * stride)
                            nc.vector.tensor_tensor(cols[:, :, 0:2], cols[:, :, 0:2], gts[l][:, k0 : k0 + gn, :], add)
                            # wh: (2 sig)^2 A = sig^2 * 4A
                            nc.vector.tensor_tensor(cols[:, :, 2:4], cols[:, :, 2:4], cols[:, :, 2:4], mult)
                            nc.vector.tensor_tensor(
                                cols[:, :, 2:4], cols[:, :, 2:4],
                                ancht[:, l, a, :][:, None, :].to_broadcast([128, gn, 2]), mult,
                            )
                            # h3d bins+orient raw copy
                            nc.vector.tensor_copy(cols[:, :, 8:14], tpt[:, :gn, c0 + 8 : c0 + 14])
                            # orient norm^2 -> norm buffer
                            sqt = scpool.tile([128, 10, 4], f32, tag="sqt")
                            nc.vector.tensor_tensor(sqt[:, :gn, :], cols[:, :, 10:14], cols[:, :, 10:14], mult)
                            sq4 = sqt[:, :gn, :].rearrange("p g (j t) -> p g j t", t=2)
                            noff = _NBOFF[l * 3 + a]
                            nc.vector.tensor_tensor(
                                nb[:, noff + k0 : noff + k0 + gn, :], sq4[:, :, :, 0], sq4[:, :, :, 1], add
                            )
                            # dims: sigmoid * 2 - 1
                            nc.scalar.activation(cols[:, :, 14:17], tpt[:, :gn, c0 + 14 : c0 + 17], sig)
                            nc.vector.tensor_scalar(cols[:, :, 14:17], cols[:, :, 14:17], 2.0, -1.0, mult, add)

                def _emit_tail(l=l, S=S, nchk=nchk):
                    if _stage != "all":
                        return
                    loff = _NBOFF[l * 3]
                    nbl = nb[:, loff : loff + 3 * nchk, :]
                    nc.scalar.activation(nbl, nbl, mybir.ActivationFunctionType.Sqrt, bias=epst[:])
                    nc.vector.reciprocal(nbl, nbl)
                    # full partitions covering whole rows: p*nchk+k < S
                    pfull = S // nchk  # 128, 123, 100
                    for a in range(3):
                        noff = _NBOFF[l * 3 + a]
                        ori = st[l][a][:, :, 10:14].rearrange("p k (j t) -> p k j t", t=2)
                        rinv = nb[:, noff : noff + nchk, :][:, :, :, None].to_broadcast(
                            [128, nchk, 2, 2]
                        )
                        nc.vector.tensor_tensor(ori, ori, rinv, mult)
                        row0 = _ROW0[l] + a * S
                        nc.scalar.dma_start(
                            out[row0 : row0 + pfull * nchk, :].rearrange(
                                "(p k) c -> p k c", k=nchk
                            ),
                            st[l][a][0:pfull, :, :],
                        )
                        rem = S - pfull * nchk  # 0, 1, 0
                        if rem:
                            nc.scalar.dma_start(
                                out[row0 + pfull * nchk : row0 + S, :],
                                st[l][a][pfull : pfull + 1, 0:rem, :],
                            )

                # defer transpose+decode past the next level's conv; defer
                # tails one more step so stores land under later compute
                pending.append(_trans_decode)
                if len(pending) > 1:
                    pending.pop(0)()
                if len(pending_tails) > 0:
                    pending_tails.pop(0)()
                pending_tails.append(_emit_tail)
                if l == 2:
                    for fn in pending:
                        fn()
                    pending.clear()
                    for fn in pending_tails:
                        fn()
                    pending_tails.clear()

    nc.finalize()
    return nc


def _pack_inputs(inputs):
    """Host-side packing: compose 1x1 heads into the 3x3 convs, pad
    activations, cast everything the PE touches to fp16."""
    f16 = np.float16
    shared = {}
    percore = [dict() for _ in range(BS)]
    bias_flags = []
    cblob = np.zeros((128, _CB_TOT), np.float32)
    for l, (C, H, W, stride, W2, _) in enumerate(LEVELS):
        Q = C // 128
        S = H * W
        nchk = _NCHK[l]
        f = np.asarray(inputs[f"f{l}"])
        fpad = np.zeros((BS, C, H + 2, W2), f16)
        fpad[:, :, 1 : H + 1, 1 : W + 1] = f.astype(f16)
        fp = np.ascontiguousarray(
            fpad.reshape(BS, Q, 128, H + 2, W2).transpose(0, 2, 1, 3, 4)
        )
        for bcore in range(BS):
            percore[bcore][f"f{l}p"] = fp[bcore]

        # compose the 1x1 head into the 3x3 conv: Wc[o,i,ky,kx] =
        # sum_c wb[o,c] * wa[c,i,ky,kx]; channel order per anchor a is
        # [8 x 2d-head | 9 x 3d-head] matching the NOUT=17 staging layout
        w2a = np.asarray(inputs[f"w2a{l}"]).astype(np.float64)
        w2b = np.asarray(inputs[f"w2b{l}"])[:, :, 0, 0].astype(np.float64)
        w3a = np.asarray(inputs[f"w3a{l}"]).astype(np.float64)
        w3b = np.asarray(inputs[f"w3b{l}"])[:, :, 0, 0].astype(np.float64)
        wc2 = np.einsum("oc,cikl->oikl", w2b, w2a)  # [24, C, 3, 3]
        wc3 = np.einsum("oc,cikl->oikl", w3b, w3a)  # [27, C, 3, 3]
        wc = np.zeros((NCH, C, 3, 3), np.float64)
        for a in range(3):
            wc[a * NOUT : a * NOUT + 8] = wc2[a * 8 : a * 8 + 8]
            wc[a * NOUT + 8 : a * NOUT + 17] = wc3[a * 9 : a * 9 + 9]
        shared[f"wa{l}"] = np.ascontiguousarray(
            wc.reshape(NCH, Q, 128, 9).transpose(2, 1, 3, 0).astype(f16)
        )

        # composed bias: bc[o] = wb[o,:] @ ba + bb[o]
        b2a = np.asarray(inputs[f"b2a{l}"]).astype(np.float64)
        b2b = np.asarray(inputs[f"b2b{l}"]).astype(np.float64)
        b3a = np.asarray(inputs[f"b3a{l}"]).astype(np.float64)
        b3b = np.asarray(inputs[f"b3b{l}"]).astype(np.float64)
        bc2 = w2b @ b2a + b2b
        bc3 = w3b @ b3a + b3b
        bc = np.zeros((NCH,), np.float64)
        for a in range(3):
            bc[a * NOUT : a * NOUT + 8] = bc2[a * 8 : a * 8 + 8]
            bc[a * NOUT + 8 : a * NOUT + 17] = bc3[a * 9 : a * 9 + 9]
        bias_flags.append(bool(np.any(bc != 0)))
        cblob[0:NCH, _CB_BIAS + l] = bc
        cblob[64 : 64 + NCH, _CB_BIAS + l] = bc

        # grid for pixel index = p*nchk + k (partition-major pixel layout)
        p = np.arange(128)[:, None]
        k = np.arange(nchk)[None, :]
        px = p * nchk + k  # [128, nchk]
        gx = np.where(px < S, px % W, 0.0).astype(np.float32)
        gy = np.where(px < S, px // W, 0.0).astype(np.float32)
        g = np.stack([(gx - 0.5) * stride, (gy - 0.5) * stride], -1)  # [128,nchk,2]
        cblob[:, _CB_GRID[l] : _CB_GRID[l] + 2 * nchk] = g.reshape(128, -1)

    cblob[:, _CB_ANCH : _CB_ANCH + 18] = (4.0 * ANCHORS).reshape(1, 18)
    idx = np.arange(NCH)
    cblob[idx, _CB_IDENT + idx] = 1.0
    cblob[64 + idx, _CB_IDENT + idx] = 1.0
    shared["cblob"] = cblob

    in_maps = []
    for bcore in range(BS):
        m = dict(shared)
        m.update(percore[bcore])
        in_maps.append(m)
    return in_maps, tuple(bias_flags)


def _get_program(bias_flags):
    if bias_flags not in _PROGRAM_CACHE:
        _PROGRAM_CACHE[bias_flags] = _build_program(bias_flags)
    return _PROGRAM_CACHE[bias_flags]


def _run(inputs, trace=False):
    from concourse.bass_utils import run_bass_kernel_spmd

    in_maps, bias_flags = _pack_inputs(inputs)
    nc = _get_program(bias_flags)
    res = run_bass_kernel_spmd(
        nc, in_maps, core_ids=list(range(NCORES)), trace=trace
    )
    outp = np.stack([res.results[i]["out"] for i in range(NCORES)]).astype(np.float32)
    return outp, res


def kernel(**inputs) -> np.ndarray:
    outp, _ = _run(inputs, trace=False)
    return outp
